# revision 33
# baseline (speedup 1.0000x reference)
"""Trainium2 Bass kernel for a ViT-style EncoderBlock.

Problem: B=4, N=2048, D=768, H=12 heads (hd=64), FFN hidden 3072, fp32.
  y = x + proj(attn(LN1(x))) ;  out = y + fc2(gelu(fc1(LN2(y))))

Sharding (8 cores, zero communication): core c handles batch b=c//2 and
query-half s=c%2 (1024 query rows).  Each core receives the full batch-b
sequence (2048 rows) with its own query rows permuted to the front, computes
K/V over all 2048 rows, attention/FFN for its 1024 rows, and returns its
[1024, 768] slice of the output.  Host reassembles.

Precision: QKV/proj run fp8e4 DoubleRow (weights host-prescaled x8; the
attention branch output is ~30x smaller than the residual, so fp8 error
there is diluted).  Scores stay bf16; exp(scale*s-4) is stored fp8 for the
DoubleRow AV matmul with the softmax denominator folded in as a ones column
of V.  The FFN stays bf16 (fp8 there would blow the 2e-2 budget).

Schedule: phases are software-pipelined so the ACT-bound softmax overlaps
PE-heavy GEMMs — the QKV tail (c>=2) is emitted inside attention ch=0, and
proj+LN2+fc1 of the first query half inside attention ch=1.  LN rsqrt uses
Ln+Exp (same ACT table set as the softmax exp); gelu of the first half is
deferred to the tail so the kernel performs exactly one table-set switch.
"""

import sys

if "/opt/trn_rl_repo" not in sys.path:
    sys.path.insert(0, "/opt/trn_rl_repo")

import numpy as np

B, N, D = 4, 2048, 768
H, HD = 12, 64
HID = 4 * D
NQ = N // 2  # query rows per core
SCALE = HD ** -0.5
EPS = 1e-5

P = 128
DT = D // P          # 6 d-tiles
TP = DT // 2         # 3 d-tile pairs (DoubleRow)
NQT = NQ // P        # 8 query tiles
NMT = N // P         # 16 kv tiles
HIDT = HID // P      # 24 hidden tiles
VW = HD + 1          # 65: V plus ones column
VP = 80              # padded V row stride (multiple of 16 for DoubleRow APs)
ESB = -4.0           # exp bias shift: es = exp(SCALE*s - 4) keeps fp8 < 240
W8C = 0.125          # compensation for host-side x8 fp8 weight prescale

INPUT_NAMES = (
    "ln1_g", "ln1_b", "qkv_w", "proj_w", "proj_b",
    "ln2_g", "ln2_b", "fc1_w", "fc1_b", "fc2_w", "fc2_b",
)


def _encoder_body(tc, out_ap, aps):
    import concourse.bass as bass
    from concourse import mybir
    from concourse.masks import make_identity

    nc = tc.nc
    f32 = mybir.dt.float32
    f32r = mybir.dt.float32r
    bf16 = mybir.dt.bfloat16
    f8 = mybir.dt.float8e4
    AF = mybir.ActivationFunctionType
    OP = mybir.AluOpType
    DR = mybir.MatmulPerfMode.DoubleRow

    def mm(psum, lhsT, rhs, start, stop):
        nc.tensor.matmul(psum, lhsT, rhs, start=start, stop=stop)

    def mmdr(psum, lhsT, rhs, start, stop):
        nc.tensor.matmul(psum, lhsT, rhs, start=start, stop=stop, perf_mode=DR)

    x = aps["x"]

    # ---------------- constants ----------------
    consts = tc.alloc_tile_pool(name="consts", bufs=1)
    ident = consts.tile([P, P], f32, name="ident")
    make_identity(nc, ident)
    identb = consts.tile([P, P], bf16, name="identb")
    nc.vector.tensor_copy(identb, ident)
    g1c = consts.tile([P, DT], f32, name="g1c")
    b1c = consts.tile([P, DT], f32, name="b1c")
    g2c = consts.tile([P, DT], f32, name="g2c")
    b2c = consts.tile([P, DT], f32, name="b2c")
    f1bc = consts.tile([P, HIDT], f32, name="f1bc")
    nc.gpsimd.dma_start(out=g1c, in_=aps["ln1_g"].rearrange("(t p) -> p t", p=P))
    nc.gpsimd.dma_start(out=b1c, in_=aps["ln1_b"].rearrange("(t p) -> p t", p=P))
    nc.gpsimd.dma_start(out=g2c, in_=aps["ln2_g"].rearrange("(t p) -> p t", p=P))
    nc.gpsimd.dma_start(out=b2c, in_=aps["ln2_b"].rearrange("(t p) -> p t", p=P))
    nc.gpsimd.dma_start(out=f1bc, in_=aps["fc1_b"].rearrange("(t p) -> p t", p=P))
    pjb = consts.tile([P, D], f32, name="pjb")
    f2b = consts.tile([P, D], f32, name="f2b")
    nc.gpsimd.dma_start(out=pjb, in_=aps["proj_b"].partition_broadcast(P))
    nc.gpsimd.dma_start(out=f2b, in_=aps["fc2_b"].partition_broadcast(P))
    ones_f = consts.tile([P, HD], f32, name="ones_f")
    nc.vector.memset(ones_f, 1.0)
    esbc = consts.tile([P, 1], f32, name="esbc")
    nc.vector.memset(esbc, ESB)
    # 8.0 column: the OT normalizer matmul broadcasts 8*recip so OT = 8*o
    # lands in fp8 normal range; the 1/64 (8 from w8, 8 from OT) is
    # compensated in the proj psum readout.
    eights_f = consts.tile([1, HD], f32, name="eights_f")
    nc.vector.memset(eights_f, 8.0)
    ones64 = consts.tile([1, HD], f32r, name="ones64")
    nc.vector.tensor_copy(ones64, eights_f)

    def _ln(small, work, x_sl):
        """Row LN of x_sl [128, D] -> normalized bf16 tile (no g/b; applied
        post-transpose).  rsqrt via Ln+Exp: stays in the exp table set."""
        st = small.tile([P, 2, 6], f32, name="ln_st")
        for g in range(2):
            nc.vector.bn_stats(st[:, g, :], x_sl[:, g * 384:(g + 1) * 384])
        mv = small.tile([P, 2], f32, name="ln_mv")
        nc.vector.bn_aggr(mv, st)
        ve = small.tile([P, 1], f32, name="ln_ve")
        nc.vector.tensor_scalar_add(ve, mv[:, 1:2], EPS)
        lv = small.tile([P, 1], f32, name="ln_lv")
        nc.scalar.activation(lv, ve, AF.Ln)
        rs = small.tile([P, 1], f32, name="ln_rs")
        nc.scalar.activation(rs, lv, AF.Exp, scale=-0.5)
        nmr = small.tile([P, 1], f32, name="ln_nmr")
        nc.vector.tensor_scalar(nmr, mv[:, 0:1], rs, -1.0, OP.mult, OP.mult)
        h = work.tile([P, D], bf16, name="ln_h")
        nc.vector.tensor_scalar(h, x_sl, rs, nmr, OP.mult, OP.add)
        return h

    # persistent tiles (right side)
    f1wp = tc.alloc_tile_pool(name="f1w", bufs=1, side="right")
    w1a = f1wp.tile([P, DT, HID], bf16, name="w1a")
    res1p = tc.alloc_tile_pool(name="res1p", bufs=1, side="right")
    res1 = res1p.tile([P, NQT, D], f32, name="res1")
    x2Tp = tc.alloc_tile_pool(name="x2Tp", bufs=1, side="right")
    x2T = x2Tp.tile([P, DT, NQ], bf16, name="x2T")
    pjwp = tc.alloc_tile_pool(name="pjwp", bufs=1, side="right")
    pjw = pjwp.tile([HD, H // 2, 2, D], f8, name="pjw")

    # persistent tiles (left side)
    qkv = tc.alloc_tile_pool(name="qkv", bufs=1)
    qT = qkv.tile([P, DT, NQ], bf16, name="qT")       # [qcol, nq]
    kT = qkv.tile([P, DT, N], bf16, name="kT")        # [kcol, m]
    # V in fp8 for DoubleRow AV: [m, mt-pair, mt-parity, h, 80(pad of 65)]
    V4 = qkv.tile([P, NMT // 2, 2, H, VP], f8, name="V4")
    hTp = tc.alloc_tile_pool(name="hTp", bufs=1)
    hT = hTp.tile([P, TP, 2, N], f8, name="hT")       # LN1(x)^T, t-subtile pairs

    lp = nc.allow_low_precision(reason="fp8/bf16 activations throughout")
    lp.__enter__()

    # ============ phase A : LN1 -> hT ; K,Q c-groups {0,1}; V cols 0:512 ====
    QN = 4
    wpool = tc.alloc_tile_pool(name="wqkv", bufs=1)
    wq = wpool.tile([P, TP, 2, D], f8, name="wq", tag="wq")
    wk = wpool.tile([P, TP, 2, D], f8, name="wk", tag="wk")
    wv = wpool.tile([P, TP, 2, D], f8, name="wv", tag="wv")
    nc.gpsimd.dma_start(
        out=wq,
        in_=aps["qkv_w"][:, 0:D].rearrange("(a b p) c -> p a b c", a=TP, b=2),
    )
    nc.gpsimd.dma_start(
        out=wk,
        in_=aps["qkv_w"][:, D:2 * D].rearrange("(a b p) c -> p a b c", a=TP, b=2),
    )
    nc.gpsimd.dma_start(
        out=wv,
        in_=aps["qkv_w"][:, 2 * D:3 * D].rearrange(
            "(a b p) c -> p a b c", a=TP, b=2),
    )
    nc.gpsimd.dma_start(out=w1a, in_=aps["fc1_w"].rearrange("(t p) c -> p t c", p=P))
    nc.gpsimd.dma_start(
        out=pjw, in_=aps["proj_w"].rearrange("(a b p) d -> p a b d", a=H // 2, b=2)
    )
    with tc.tile_pool(name="p1work", bufs=6) as work, \
         tc.tile_pool(name="p1small", bufs=8) as small, \
         tc.tile_pool(name="p1psum", bufs=2, space="PSUM") as psT, \
         tc.tile_pool(name="p2psum", bufs=2, space="PSUM") as psQ, \
         tc.tile_pool(name="pvpsum", bufs=2, space="PSUM") as psV:

        def ln_quarter(q):
            for i in range(q * QN, (q + 1) * QN):
                xt = work.tile([P, D], f32, name="xt", tag="xt")
                nc.sync.dma_start(out=xt, in_=x[i * P:(i + 1) * P, :])
                h = _ln(small, work, xt)
                for t in range(DT):
                    ps = psT.tile([P, P], bf16, name="trps")
                    nc.tensor.transpose(ps, h[:, t * P:(t + 1) * P], identb)
                    nc.vector.tensor_scalar(
                        hT[:, t // 2, t % 2, i * P:(i + 1) * P], ps,
                        g1c[:, t:t + 1], b1c[:, t:t + 1], OP.mult, OP.add,
                    )

        def kq_chunk(wt, dst, c, cs):
            ps = psQ.tile([P, 512], f32, name="kqps", tag="ps")
            for tp in range(TP):
                mmdr(ps, wt[:, tp, :, c * P:(c + 1) * P], hT[:, tp, :, cs],
                     tp == 0, tp == TP - 1)
            nc.scalar.activation(dst[:, c, cs], ps, AF.Identity, scale=W8C)

        def v_chunk(psV_, wvt, mt, lo):
            """V columns lo*512 ..+w (heads 8lo..) for key block mt."""
            w = 512 if lo == 0 else 256
            psv = psV_.tile([P, 512], f32, name="vps", tag="c")
            for tp in range(TP):
                mmdr(psv[:, 0:w], hT[:, tp, :, mt * P:(mt + 1) * P],
                     wvt[:, tp, :, lo * 512:lo * 512 + w], tp == 0, tp == TP - 1)
            nc.scalar.activation(
                V4[:, mt // 2, mt % 2, 8 * lo:8 * lo + w // HD, 0:HD],
                psv[:, 0:w].rearrange("p (a b) -> p a b", b=HD),
                AF.Identity, scale=W8C,
            )
            if lo == 0:
                nc.vector.tensor_copy(
                    V4[:, mt // 2, mt % 2, :, HD:VW],
                    ones_f[:, 0:H].rearrange("p (a b) -> p a b", b=1),
                )

        ln_quarter(0)
        for q in range(4):
            cs = slice(q * 512, (q + 1) * 512)
            for c in (0, 1):
                kq_chunk(wk, kT, c, cs)
            if q < 2:
                for c in (0, 1):
                    kq_chunk(wq, qT, c, cs)
            for mt in range(q * QN, (q + 1) * QN):
                v_chunk(psV, wv, mt, 0)
            if q + 1 < 4:
                ln_quarter(q + 1)

    # ============ attention (+ pipelined QKV tail / FFN first half) =========
    otp = tc.alloc_tile_pool(name="otp", bufs=1, side="right")
    OT = otp.tile([HD, H // 2, 2, NQ], f8, name="OT")  # 8*o^T, head pairs

    def attention_j(j, ch, esp, asmall, psS, psB, psO):
        hA, hB = 2 * j, 2 * j + 1
        kTa, kTb = kT[0:HD, j, :], kT[HD:P, j, :]
        qTa, qTb = qT[0:HD, j, :], qT[HD:P, j, :]
        cs = slice(ch * 512, (ch + 1) * 512)
        poA = psO.tile([VW, 512], f32, name="poA", tag="po")
        poB = psO.tile([VW, 512], f32, name="poB", tag="po")
        for mtp in range(NMT // 2):
            es2 = esp.tile([P, 2, 1024], f8, name="es2")
            ps = psS.tile([P, 2, 1024], f32, name="sps")
            for par in range(2):
                mt = 2 * mtp + par
                msl = slice(mt * P, (mt + 1) * P)
                # two heads on the two 64-row halves of the PE array
                mm(ps[:, par, 0:512], kTa[:, msl], qTa[:, cs], True, True)
                mm(ps[:, par, 512:1024], kTb[:, msl], qTb[:, cs], True, True)
            nc.scalar.activation(es2, ps, AF.Exp, scale=SCALE, bias=esbc)
            last = mtp == NMT // 2 - 1
            mmdr(poA, V4[:, mtp, :, hA, 0:VW], es2[:, :, 0:512], mtp == 0, last)
            mmdr(poB, V4[:, mtp, :, hB, 0:VW], es2[:, :, 512:1024], mtp == 0, last)
        posbA = asmall.tile([VW, 512], f32, name="posbA", tag="posbA", bufs=2)
        nc.vector.tensor_copy(posbA, poA)
        posbB = asmall.tile([VW, 512], f32, name="posbB", tag="posbB", bufs=2)
        nc.vector.tensor_copy(posbB, poB)
        for h, posb in ((hA, posbA), (hB, posbB)):
            rd = asmall.tile([1, 512], f32r, name="rd", bufs=2)
            nc.vector.reciprocal(rd, posb[HD:VW, :])
            rb = psB.tile([HD, 512], f32, name="rb")
            mm(rb, ones64, rd, True, True)
            ots = OT[:, h // 2, h % 2, cs]
            nc.vector.tensor_tensor(ots, posb[0:HD, :], rb, OP.mult)

    def proj_unit(i, work4, small4, psC):
        """proj + residual + LN2 + x2T for query block i (1-bank psum tag)."""
        xr = work4.tile([P, D], f32, name="xr", tag="xr")
        nc.sync.dma_start(out=xr, in_=x[i * P:(i + 1) * P, :])
        r1 = res1[:, i, :]
        for pslo, w in ((0, 512), (512, 256)):
            psp = psC.tile([P, 512], f32, name="psp", tag="c")
            for hp in range(H // 2):
                mmdr(psp[:, 0:w], OT[:, hp, :, i * P:(i + 1) * P],
                     pjw[:, hp, :, pslo:pslo + w], hp == 0, hp == H // 2 - 1)
            # psp holds 64*proj-out (8 from w8, 8 from OT=8*o)
            sl = slice(pslo, pslo + w)
            nc.vector.tensor_scalar_mul(r1[:, sl], psp[:, 0:w], 1.0 / 64.0)
            nc.vector.tensor_add(r1[:, sl], r1[:, sl], xr[:, sl])
            nc.vector.tensor_add(r1[:, sl], r1[:, sl], pjb[:, sl])
        h2 = _ln(small4, work4, r1)
        nc.vector.tensor_add(r1, r1, f2b)  # fc2 bias pre-add (after LN2 read)
        for t in range(DT):
            ps = psC.tile([P, P], bf16, name="trps4", tag="c")
            nc.tensor.transpose(ps, h2[:, t * P:(t + 1) * P], identb)
            nc.vector.tensor_scalar(
                x2T[:, t, i * P:(i + 1) * P], ps,
                g2c[:, t:t + 1], b2c[:, t:t + 1], OP.mult, OP.add,
            )

    def fc1_unit(hc, ch, psC, h1t):
        """fc1 for hidden block hc over query chunk ch -> raw+bias -> h1t."""
        csl = slice(ch * 512, (ch + 1) * 512)
        psf = psC.tile([P, 512], f32, name="psf", tag="c")
        for t in range(DT):
            mm(psf, w1a[:, t, hc * P:(hc + 1) * P], x2T[:, t, csl],
               t == 0, t == DT - 1)
        # defer gelu (table set!): store raw + bias, gelu in-place in the tail
        nc.vector.tensor_scalar(
            h1t[:, hc, :], psf, f1bc[:, hc:hc + 1], None, OP.add)

    # ---- ch 0: attention + QKV tail ----
    with tc.tile_pool(name="a_es", bufs=4) as esp, \
         tc.tile_pool(name="a_small", bufs=4) as asmall, \
         tc.tile_pool(name="a_psS", bufs=1, space="PSUM") as psS, \
         tc.tile_pool(name="a_psB", bufs=1, space="PSUM") as psB, \
         tc.tile_pool(name="a_psO", bufs=2, space="PSUM") as psO, \
         tc.tile_pool(name="a_psQ", bufs=1, space="PSUM") as psQB:

        def kq_tail(c):
            for q in range(4):
                cs = slice(q * 512, (q + 1) * 512)
                ps = psQB.tile([P, 512], f32, name="kps2", tag="c")
                for tp in range(TP):
                    mmdr(ps, wk[:, tp, :, c * P:(c + 1) * P], hT[:, tp, :, cs],
                         tp == 0, tp == TP - 1)
                nc.scalar.activation(kT[:, c, cs], ps, AF.Identity, scale=W8C)
                if q < 2:
                    ps2_ = psQB.tile([P, 512], f32, name="qps2", tag="c")
                    for tp in range(TP):
                        mmdr(ps2_, wq[:, tp, :, c * P:(c + 1) * P],
                             hT[:, tp, :, cs], tp == 0, tp == TP - 1)
                    nc.scalar.activation(qT[:, c, cs], ps2_, AF.Identity, scale=W8C)

        for j in range(H // 2):
            attention_j(j, 0, esp, asmall, psS, psB, psO)
            if j == 0:
                kq_tail(2)
            elif j == 1:
                kq_tail(3)
            elif j == 2:
                for mt in range(0, 8):
                    v_chunk(psQB, wv, mt, 1)
            elif j == 3:
                kq_tail(4)
                for mt in range(8, 16):
                    v_chunk(psQB, wv, mt, 1)
            elif j == 4:
                kq_tail(5)

    wpool.release()
    hTp.release()
    h1ap = tc.alloc_tile_pool(name="h1a", bufs=1, side="right")
    h1a = h1ap.tile([P, HIDT, 512], bf16, name="h1a")  # fc1 raw+bias (pre-gelu)

    # ---- ch 1: attention + proj/LN2/fc1 of first query half ----
    with tc.tile_pool(name="b_es", bufs=4) as esp, \
         tc.tile_pool(name="b_small", bufs=4) as asmall, \
         tc.tile_pool(name="b_work", bufs=2) as work4, \
         tc.tile_pool(name="b_lns", bufs=8) as small4, \
         tc.tile_pool(name="b_psS", bufs=1, space="PSUM") as psS, \
         tc.tile_pool(name="b_psB", bufs=1, space="PSUM") as psB, \
         tc.tile_pool(name="b_psO", bufs=2, space="PSUM") as psO, \
         tc.tile_pool(name="b_psC", bufs=1, space="PSUM") as psC:
        for j in range(H // 2):
            attention_j(j, 1, esp, asmall, psS, psB, psO)
            if j < 2:
                for i in (2 * j, 2 * j + 1):
                    proj_unit(i, work4, small4, psC)
            else:
                for hc in range(6 * (j - 2), 6 * (j - 1)):
                    fc1_unit(hc, 0, psC, h1a)

    qkv.release()

    # ============ tail: gelu(ch0), proj/LN2(ch1), fc1(ch1), fc2 ============
    f2wp = tc.alloc_tile_pool(name="f2w", bufs=1, side="right")
    w2a = f2wp.tile([P, HIDT, D], bf16, name="w2a")
    nc.gpsimd.dma_start(out=w2a, in_=aps["fc2_w"].rearrange("(j p) d -> p j d", p=P))
    h1bp = tc.alloc_tile_pool(name="h1b", bufs=1, side="right")
    h1b = h1bp.tile([P, HIDT, 512], bf16, name="h1b")

    with tc.tile_pool(name="t_work", bufs=4) as work4, \
         tc.tile_pool(name="t_lns", bufs=8) as small4, \
         tc.tile_pool(name="t_psP", bufs=1, space="PSUM") as psP, \
         tc.tile_pool(name="t_psF", bufs=2, space="PSUM") as psF, \
         tc.tile_pool(name="t_ps2", bufs=2, space="PSUM") as ps2:
        for i in range(4, NQT):
            proj_unit(i, work4, small4, psP)
        # deferred gelu for ch0 (bias already added), batched 4 blocks/call;
        # emitted after all tail Ln/Exp so the gelu table set loads once
        for hc4 in range(0, HIDT, 4):
            nc.scalar.activation(h1a[:, hc4:hc4 + 4, :], h1a[:, hc4:hc4 + 4, :],
                                 AF.Gelu)
        for hc in range(HIDT):
            csl = slice(512, 1024)
            psf = psF.tile([P, 512], f32, name="psf1")
            for t in range(DT):
                mm(psf, w1a[:, t, hc * P:(hc + 1) * P], x2T[:, t, csl],
                   t == 0, t == DT - 1)
            nc.scalar.activation(h1b[:, hc, :], psf, AF.Gelu,
                                 bias=f1bc[:, hc:hc + 1])

        def fc2_unit(i2, ch, h1t):
            i = ch * 4 + i2
            psq = ps2.tile([P, D], f32, name="psq")
            for jj in range(HIDT):
                mm(psq[:, 0:512], h1t[:, jj, i2 * P:(i2 + 1) * P],
                   w2a[:, jj, 0:512], jj == 0, jj == HIDT - 1)
            for jj in range(HIDT):
                mm(psq[:, 512:768], h1t[:, jj, i2 * P:(i2 + 1) * P],
                   w2a[:, jj, 512:768], jj == 0, jj == HIDT - 1)
            nc.vector.tensor_add(res1[:, i, :], res1[:, i, :], psq)
            nc.sync.dma_start(out=out_ap[i * P:(i + 1) * P, :], in_=res1[:, i, :])

        for i2 in range(4):
            fc2_unit(i2, 0, h1a)
        for i2 in range(4):
            fc2_unit(i2, 1, h1b)

    lp.__exit__(None, None, None)
    h1bp.release()
    f2wp.release()
    h1ap.release()
    otp.release()
    pjwp.release()
    x2Tp.release()
    res1p.release()
    f1wp.release()
    consts.release()


def build_nc(hoist_waits=True):
    import concourse.bass as bass
    import concourse.tile as tile
    from concourse import mybir

    f32 = mybir.dt.float32
    f8 = mybir.dt.float8e4
    bf16 = mybir.dt.bfloat16
    nc = bass.Bass("TRN2", target_bir_lowering=False, debug=False)
    aps = {"x": nc.dram_tensor("x", [N, D], f32, kind="ExternalInput").ap()}
    shapes = {
        "ln1_g": [D], "ln1_b": [D], "qkv_w": [D, 3 * D],
        "proj_w": [D, D], "proj_b": [D], "ln2_g": [D], "ln2_b": [D],
        "fc1_w": [D, HID], "fc1_b": [HID], "fc2_w": [HID, D], "fc2_b": [D],
    }
    dts = {"qkv_w": f8, "proj_w": f8, "fc1_w": bf16, "fc2_w": bf16}
    for name in INPUT_NAMES:
        aps[name] = nc.dram_tensor(
            name, shapes[name], dts.get(name, f32), kind="ExternalInput"
        ).ap()
    out_ap = nc.dram_tensor("out", [NQ, D], f32, kind="ExternalOutput").ap()
    with tile.TileContext(nc) as tc:
        _encoder_body(tc, out_ap, aps)
    if hoist_waits:
        _hoist_matmul_waits(nc)
    return nc


def _hoist_matmul_waits(nc):
    """walrus's LW-path matmuls (transpose / fp32 / f32r self-loading) accept
    only one embedded sync-wait.  Tile can attach two (one per producer
    engine).  Hoist all-but-one onto a standalone InstEventSemaphore placed
    just before the matmul in the same engine stream."""
    from concourse import mybir

    skip = (
        mybir.InstEventSemaphore,
        mybir.InstUnconditionalBranch,
    )
    for f in nc.m.functions:
        for bb in f.blocks:
            out = []
            for ins in bb.instructions:
                si = getattr(ins, "sync_info", None)
                if (
                    si is not None
                    and si.on_wait
                    and len(si.on_wait) > 1
                    and not isinstance(ins, skip)
                ):
                    for k, wait in enumerate(si.on_wait[:-1]):
                        w = mybir.InstEventSemaphore(
                            name=f"{ins.name}-hoistwait{k}",
                            ins=[],
                            outs=[],
                        )
                        w.engine = ins.engine
                        w.sync_info = mybir.SyncInfo(on_wait=[wait], on_update=[])
                        out.append(w)
                    ins.sync_info = mybir.SyncInfo(
                        on_wait=[si.on_wait[-1]], on_update=list(si.on_update)
                    )
                out.append(ins)
            bb.instructions[:] = out


_NC_CACHE = {}
_WCAST = {}


def _prep_wcast(inputs):
    import ml_dtypes

    for k in ("qkv_w", "proj_w"):
        _WCAST[k] = (np.asarray(inputs[k], np.float32) * 8.0).astype(
            ml_dtypes.float8_e4m3)
    for k in ("fc1_w", "fc2_w"):
        _WCAST[k] = np.asarray(inputs[k], np.float32).astype(ml_dtypes.bfloat16)


def make_in_maps(inputs):
    _prep_wcast(inputs)
    in_maps = []
    for c in range(8):
        b, s = c // 2, c % 2
        xb = np.asarray(inputs["x"][b], dtype=np.float32)
        xp = xb if s == 0 else np.ascontiguousarray(
            np.concatenate([xb[NQ:], xb[:NQ]], axis=0)
        )
        m = {"x": xp}
        for k in INPUT_NAMES:
            if k in _WCAST:
                m[k] = _WCAST[k]
            else:
                m[k] = np.asarray(inputs[k], dtype=np.float32)
        in_maps.append(m)
    return in_maps


def kernel(**inputs):
    from concourse import bass_utils

    if "nc" not in _NC_CACHE:
        _NC_CACHE["nc"] = build_nc()
    nc = _NC_CACHE["nc"]
    in_maps = make_in_maps(inputs)
    res = bass_utils.run_bass_kernel_spmd(nc, in_maps, core_ids=list(range(8)))
    out = np.empty((B, N, D), np.float32)
    for c in range(8):
        b, s = c // 2, c % 2
        out[b, s * NQ:(s + 1) * NQ] = res.results[c]["out"]
    return out


if __name__ == "__main__":
    rng = np.random.default_rng(0)
    fake = {
        "x": rng.standard_normal((B, N, D), dtype=np.float32),
        "ln1_g": np.ones(D, np.float32), "ln1_b": np.zeros(D, np.float32),
        "qkv_w": (rng.standard_normal((D, 3 * D)) / np.sqrt(D)).astype(np.float32),
        "proj_w": (rng.standard_normal((D, D)) / np.sqrt(D)).astype(np.float32),
        "proj_b": np.zeros(D, np.float32),
        "ln2_g": np.ones(D, np.float32), "ln2_b": np.zeros(D, np.float32),
        "fc1_w": (rng.standard_normal((D, HID)) / np.sqrt(D)).astype(np.float32),
        "fc1_b": np.zeros(HID, np.float32),
        "fc2_w": (rng.standard_normal((HID, D)) / np.sqrt(HID)).astype(np.float32),
        "fc2_b": np.zeros(D, np.float32),
    }
    out = kernel(**fake)
    print("kernel ran, out shape", out.shape)
